# revision 1
# baseline (speedup 1.0000x reference)
"""MultiHeadCredibilityAttention TRN2 kernel.

Sharding: 8 cores = (batch b, query-half qh). Each core computes K/V for its
full batch sequence (S=2048) and attention outputs for its 1024 queries.
Output slices concatenate to the full (4, 2048, 256) result — no collectives.

Device-side layout strategy (all layouts prepared on host):
  - xT (D, S): x[b] transposed, so projections contract d on partitions.
  - K^T, Q^T computed directly (head dims on partitions) so scores^T
    (keys x queries) comes out of the PE without any on-device transpose.
  - Q^T stored head-masked (other heads zeroed) so score matmuls run with
    K=128 contraction (no PE tile-mode switching).
  - V stored token-major with a ones-column per head: the attn@V matmul
    simultaneously produces the softmax denominator (row 32 of ctx psum).
  - softmax skips max-subtraction (scores are O(1) by construction).
"""

import os
import sys

import numpy as np

sys.path.insert(0, "/opt/trn_rl_repo")

import concourse.bass as bass  # noqa: E402
import concourse.mybir as mybir  # noqa: E402
from concourse.tile import TileContext  # noqa: E402
from concourse import bass_utils  # noqa: E402

B, S, D, H, HD = 4, 2048, 256, 8, 32
SQ = S // 2  # queries per core
N_CORES = 8
F32 = mybir.dt.float32
F32R = mybir.dt.float32r
BF16 = mybir.dt.bfloat16
INV_SCALE = 1.0 / np.sqrt(HD)

# matmul input dtype per group: "f32" (slow, exact), "f32r", "bf16"
SCORE_MM = os.environ.get("K_SCORE_MM", "f32r")
ATTNV_MM = os.environ.get("K_ATTNV_MM", "f32r")
PROJ_MM = os.environ.get("K_PROJ_MM", "f32r")


def _dt(mode):
    return {"f32": F32, "f32r": F32R, "bf16": BF16}[mode]


def _np_dt(mode):
    if mode == "bf16":
        import ml_dtypes

        return np.dtype(ml_dtypes.bfloat16)
    return np.dtype(np.float32)


def split_multiwaits(nc, max_waits=1):
    """This toolchain's walrus rejects >1 sync-wait per instruction; split
    extras into preceding single-wait NOPs on the same engine."""
    n = 0
    for f in nc.m.functions:
        for bb in f.blocks:
            out = []
            for ins in bb.instructions:
                si = ins.sync_info
                if (
                    si is not None
                    and si.on_wait is not None
                    and len(si.on_wait) > max_waits
                ):
                    waits = list(si.on_wait)
                    for j, w in enumerate(waits[:-max_waits]):
                        n += 1
                        out.append(
                            mybir.InstNoOp(
                                name=f"{ins.name}-wsplit{j}",
                                opcode="NoOp",
                                engine=ins.engine,
                                sync_info=mybir.SyncInfo(on_wait=[w], on_update=[]),
                            )
                        )
                    ins.sync_info = mybir.SyncInfo(
                        on_wait=waits[-max_waits:], on_update=list(si.on_update)
                    )
                out.append(ins)
            bb.instructions = out
    return n


def build_module():
    PT = _dt(PROJ_MM)
    ST = _dt(SCORE_MM)
    AT = _dt(ATTNV_MM)

    nc = bass.Bass("TRN2")
    xT_d = nc.dram_tensor("xT", [D, S], PT, kind="ExternalInput")
    xTq_d = nc.dram_tensor("xTq", [D, SQ], PT, kind="ExternalInput")
    wqT_d = nc.dram_tensor("wqT", [D, D], PT, kind="ExternalInput")
    wkT_d = nc.dram_tensor("wkT", [D, D], PT, kind="ExternalInput")
    wvT_d = nc.dram_tensor("wvT", [D, D], PT, kind="ExternalInput")
    woT_d = nc.dram_tensor("woT", [D, D], PT, kind="ExternalInput")
    bq_d = nc.dram_tensor("bq", [D, 1], F32, kind="ExternalInput")
    bk_d = nc.dram_tensor("bk", [D, 1], F32, kind="ExternalInput")
    bvb_d = nc.dram_tensor("bvb", [128, D], F32, kind="ExternalInput")
    zq_d = nc.dram_tensor("zq", [128, SQ], ST, kind="ExternalInput")
    onesc_d = nc.dram_tensor("onesc", [128, H], AT, kind="ExternalInput")
    bob_d = nc.dram_tensor("bob", [128, D], F32, kind="ExternalInput")
    out_d = nc.dram_tensor("out", [SQ, D], F32, kind="ExternalOutput")

    NKT = S // 512  # key n-chunks for KT build
    NQT = SQ // 512  # query n-chunks
    NM = S // 128  # key tiles (attn contraction)
    NT = SQ // 128  # output token tiles

    with TileContext(nc) as tc:
        with (
            tc.tile_pool(name="const", bufs=1) as cpool,
            tc.tile_pool(name="pers", bufs=1) as pers,
        ):
            # ---- load inputs ----
            xT_sb = [
                cpool.tile([128, S], PT, tag=f"xT{d}", name=f"xT{d}")
                for d in range(2)
            ]
            xTq_sb = [
                cpool.tile([128, SQ], PT, tag=f"xTq{d}", name=f"xTq{d}")
                for d in range(2)
            ]
            w_sb = {}
            for nm, dram in (("q", wqT_d), ("k", wkT_d), ("v", wvT_d), ("o", woT_d)):
                w_sb[nm] = [
                    cpool.tile([128, D], PT, tag=f"w{nm}{d}", name=f"w{nm}{d}")
                    for d in range(2)
                ]
                for d in range(2):
                    nc.sync.dma_start(
                        out=w_sb[nm][d][:], in_=dram[d * 128 : (d + 1) * 128, :]
                    )
            for d in range(2):
                nc.sync.dma_start(out=xT_sb[d][:], in_=xT_d[d * 128 : (d + 1) * 128, :])
                nc.sync.dma_start(
                    out=xTq_sb[d][:], in_=xTq_d[d * 128 : (d + 1) * 128, :]
                )
            bq_sb = [
                cpool.tile([128, 1], F32, tag=f"bq{d}", name=f"bq{d}") for d in range(2)
            ]
            bk_sb = [
                cpool.tile([128, 1], F32, tag=f"bk{d}", name=f"bk{d}") for d in range(2)
            ]
            for d in range(2):
                nc.sync.dma_start(out=bq_sb[d][:], in_=bq_d[d * 128 : (d + 1) * 128, :])
                nc.sync.dma_start(out=bk_sb[d][:], in_=bk_d[d * 128 : (d + 1) * 128, :])
            bvb_sb = cpool.tile([128, D], F32, tag="bvb", name="bvb")
            bob_sb = cpool.tile([128, D], F32, tag="bob", name="bob")
            nc.sync.dma_start(out=bvb_sb[:], in_=bvb_d[:])
            nc.sync.dma_start(out=bob_sb[:], in_=bob_d[:])

            # ---- persistent intermediates ----
            KT_sb = [
                pers.tile([128, S], ST, tag=f"KT{d}", name=f"KT{d}") for d in range(2)
            ]
            QTm_sb = [
                pers.tile([128, SQ], ST, tag=f"QTm{h}", name=f"QTm{h}")
                for h in range(H)
            ]
            VA_sb = [
                pers.tile([128, H * (HD + 1)], AT, tag=f"VA{m}", name=f"VA{m}")
                for m in range(NM)
            ]
            ctxT_sb = [
                pers.tile([128, SQ], PT, tag=f"ctxT{d}", name=f"ctxT{d}")
                for d in range(2)
            ]
            craw_sb = [
                pers.tile([HD + 1, SQ], F32, tag=f"craw{h}", name=f"craw{h}")
                for h in range(H)
            ]

            # zero the masked-Q tiles (other heads' rows must be 0)
            for h in range(H):
                nc.sync.dma_start(out=QTm_sb[h][:], in_=zq_d[:])

            # ---- projections ----
            with tc.tile_pool(name="ps1", bufs=2, space="PSUM") as ps1:
                # K^T and Q^T (head dims on partitions); d-outer so the
                # stationary weight chunk is reused across n-chunks.
                for half in range(2):
                    pks = [
                        ps1.tile([128, 512], F32, tag=f"pk{kn}", name=f"pk{kn}", bufs=1)
                        for kn in range(NKT)
                    ]
                    pqs = [
                        ps1.tile([128, 512], F32, tag=f"pq{qn}", name=f"pq{qn}", bufs=1)
                        for qn in range(NQT)
                    ]
                    for d in range(2):
                        for kn in range(NKT):
                            nc.tensor.matmul(
                                pks[kn][:],
                                lhsT=w_sb["k"][d][:, half * 128 : (half + 1) * 128],
                                rhs=xT_sb[d][:, kn * 512 : (kn + 1) * 512],
                                start=(d == 0),
                                stop=(d == 1),
                            )
                        for qn in range(NQT):
                            nc.tensor.matmul(
                                pqs[qn][:],
                                lhsT=w_sb["q"][d][:, half * 128 : (half + 1) * 128],
                                rhs=xTq_sb[d][:, qn * 512 : (qn + 1) * 512],
                                start=(d == 0),
                                stop=(d == 1),
                            )
                    for kn in range(NKT):
                        nc.vector.tensor_scalar_add(
                            KT_sb[half][:, kn * 512 : (kn + 1) * 512],
                            pks[kn][:],
                            bk_sb[half][:, 0:1],
                        )
                    for qn in range(NQT):
                        for hh in range(4):
                            h = half * 4 + hh
                            nc.vector.tensor_scalar_add(
                                QTm_sb[h][
                                    32 * hh : 32 * (hh + 1), qn * 512 : (qn + 1) * 512
                                ],
                                pqs[qn][32 * hh : 32 * (hh + 1), :],
                                bq_sb[half][32 * hh : 32 * (hh + 1), 0:1],
                            )
                # V (token-major) with ones column per head
                for m in range(NM):
                    pv = ps1.tile([128, D], F32, tag="pv", name="pv")
                    for d in range(2):
                        nc.tensor.matmul(
                            pv[:],
                            lhsT=xT_sb[d][:, m * 128 : (m + 1) * 128],
                            rhs=w_sb["v"][d][:],
                            start=(d == 0),
                            stop=(d == 1),
                        )
                    va = VA_sb[m][:].rearrange("p (h c) -> p h c", c=HD + 1)
                    nc.vector.tensor_add(
                        va[:, :, 0:HD],
                        pv[:].rearrange("p (h c) -> p h c", c=HD),
                        bvb_sb[:].rearrange("p (h c) -> p h c", c=HD),
                    )
                    nc.sync.dma_start(
                        out=va[:, :, HD : HD + 1], in_=onesc_d[:, :, None]
                    )

            # ---- attention: head pairs, flash-style over key tiles ----
            with (
                tc.tile_pool(name="ps_s", bufs=2, space="PSUM") as ps_s,
                tc.tile_pool(name="ps_ctx", bufs=2, space="PSUM") as ps_ctx,
                tc.tile_pool(name="expp", bufs=4) as expp,
                tc.tile_pool(name="dramp", bufs=2, space="DRAM") as dramp,
            ):
                for hg in range(H // 2):
                    heads = (2 * hg, 2 * hg + 1)
                    pctx = {
                        h: ps_ctx.tile([HD + 1, SQ], F32, tag="ctx", name="ctx")
                        for h in heads
                    }
                    for m in range(NM):
                        psc = {
                            h: ps_s.tile([128, SQ], F32, tag="s", name="s")
                            for h in heads
                        }
                        for h in heads:
                            half = h // 4
                            for qn in range(NQT):
                                nc.tensor.matmul(
                                    psc[h][:, qn * 512 : (qn + 1) * 512],
                                    lhsT=KT_sb[half][:, m * 128 : (m + 1) * 128],
                                    rhs=QTm_sb[h][:, qn * 512 : (qn + 1) * 512],
                                    start=True,
                                    stop=True,
                                )
                        ets = {}
                        for h in heads:
                            et = expp.tile([128, SQ], AT, tag="exp", name="exp")
                            nc.scalar.activation(
                                et[:],
                                psc[h][:],
                                mybir.ActivationFunctionType.Exp,
                                scale=float(INV_SCALE),
                            )
                            ets[h] = et
                        for h in heads:
                            for qn in range(NQT):
                                nc.tensor.matmul(
                                    pctx[h][:, qn * 512 : (qn + 1) * 512],
                                    lhsT=VA_sb[m][
                                        :, h * (HD + 1) : (h + 1) * (HD + 1)
                                    ],
                                    rhs=ets[h][:, qn * 512 : (qn + 1) * 512],
                                    start=(m == 0),
                                    stop=(m == NM - 1),
                                )
                    # quick PSUM evacuation: raw ctx (+denominator row) to SBUF
                    for h in heads:
                        nc.vector.tensor_copy(craw_sb[h][:], pctx[h][:])

                # batched finalize: one reciprocal over all 8 heads, then
                # per-head normalize. Denominators reshaped via DRAM bounce.
                rd8 = dramp.tile([H, SQ], F32, tag="rd8", name="rd8")
                for h in range(H):
                    nc.sync.dma_start(
                        out=rd8[h : h + 1, :], in_=craw_sb[h][HD : HD + 1, :]
                    )
                den8 = expp.tile([H, SQ], F32, tag="den8", name="den8")
                nc.sync.dma_start(out=den8[:], in_=rd8[:])
                rec8 = expp.tile([H, SQ], F32, tag="rec8", name="rec8")
                nc.vector.reciprocal(rec8[:], den8[:])
                rdr = dramp.tile([H, SQ], F32, tag="rdr", name="rdr")
                nc.sync.dma_start(out=rdr[:], in_=rec8[:])
                for h in range(H):
                    half, hh = h // 4, h % 4
                    rb = expp.tile([HD, SQ], F32, tag="rb", name="rb")
                    nc.sync.dma_start(
                        out=rb[:], in_=rdr[h : h + 1, :].to_broadcast((HD, SQ))
                    )
                    nc.vector.tensor_mul(
                        ctxT_sb[half][32 * hh : 32 * (hh + 1), :],
                        craw_sb[h][0:HD, :],
                        rb[:],
                    )

            # ---- output projection ----
            with (
                tc.tile_pool(name="ps_o", bufs=2, space="PSUM") as ps_o,
                tc.tile_pool(name="outp", bufs=2) as outp,
            ):
                for t in range(NT):
                    po = ps_o.tile([128, D], F32, tag="po", name="po")
                    for d in range(2):
                        nc.tensor.matmul(
                            po[:],
                            lhsT=ctxT_sb[d][:, t * 128 : (t + 1) * 128],
                            rhs=w_sb["o"][d][:],
                            start=(d == 0),
                            stop=(d == 1),
                        )
                    ot = outp.tile([128, D], F32, tag="ot", name="ot")
                    nc.vector.tensor_add(ot[:], po[:], bob_sb[:])
                    nc.sync.dma_start(out=out_d[t * 128 : (t + 1) * 128, :], in_=ot[:])

    split_multiwaits(nc)
    return nc


if os.environ.get("K_LDW_OPT", "0") == "1" and not getattr(
    bass_utils, "_k_ldw_patched", False
):
    _orig_run_command = bass_utils.run_command

    def _run_command_ldw(argv, **kwargs):
        argv = [
            "--enable-ldw-opt=true" if a == "--enable-ldw-opt=false" else a
            for a in argv
        ]
        return _orig_run_command(argv, **kwargs)

    bass_utils.run_command = _run_command_ldw
    bass_utils._k_ldw_patched = True


_module_cache = {}


def _get_module():
    key = (SCORE_MM, ATTNV_MM, PROJ_MM)
    if key not in _module_cache:
        _module_cache[key] = build_module()
    return _module_cache[key]


def make_in_maps(inputs):
    x = np.asarray(inputs["x"], np.float32)
    Wq, bq = np.asarray(inputs["Wq"], np.float32), np.asarray(inputs["bq"], np.float32)
    Wk, bk = np.asarray(inputs["Wk"], np.float32), np.asarray(inputs["bk"], np.float32)
    Wv, bv = np.asarray(inputs["Wv"], np.float32), np.asarray(inputs["bv"], np.float32)
    Wo, bo = np.asarray(inputs["Wo"], np.float32), np.asarray(inputs["bo"], np.float32)
    hs = np.asarray(inputs["head_scale"], np.float32)

    pdt = _np_dt(PROJ_MM)
    hs_col = np.repeat(hs, HD)  # head_scale folded into V
    shared = {
        "wqT": np.ascontiguousarray(Wq.T.astype(pdt)),
        "wkT": np.ascontiguousarray(Wk.T.astype(pdt)),
        "wvT": np.ascontiguousarray((Wv * hs_col[:, None]).T.astype(pdt)),
        "woT": np.ascontiguousarray(Wo.T.astype(pdt)),
        "bq": np.ascontiguousarray(bq.reshape(D, 1)),
        "bk": np.ascontiguousarray(bk.reshape(D, 1)),
        "bvb": np.ascontiguousarray(
            np.broadcast_to(bv * hs_col, (128, D)).astype(np.float32)
        ),
        "bob": np.ascontiguousarray(np.broadcast_to(bo, (128, D)).astype(np.float32)),
        "zq": np.zeros((128, SQ), _np_dt(SCORE_MM)),
        "onesc": np.ones((128, H), _np_dt(ATTNV_MM)),
    }
    in_maps = []
    for k in range(N_CORES):
        b, qh = k // 2, k % 2
        xT = np.ascontiguousarray(x[b].T.astype(pdt))
        in_maps.append(
            {
                "xT": xT,
                "xTq": np.ascontiguousarray(xT[:, qh * SQ : (qh + 1) * SQ]),
                **shared,
            }
        )
    return in_maps


def kernel(x, Wq, bq, Wk, bk, Wv, bv, Wo, bo, head_scale):
    in_maps = make_in_maps(
        dict(
            x=x, Wq=Wq, bq=bq, Wk=Wk, bk=bk, Wv=Wv, bv=bv, Wo=Wo, bo=bo,
            head_scale=head_scale,
        )
    )
    nc = _get_module()
    core_ids = list(range(N_CORES))
    # First execution after a fresh process attach has been observed to
    # return corrupted results on some cores; run once to warm up, then
    # use the second run (retry if it still looks corrupted).
    bass_utils.run_bass_kernel_spmd(nc, in_maps, core_ids=core_ids)
    for _ in range(3):
        res = bass_utils.run_bass_kernel_spmd(nc, in_maps, core_ids=core_ids)
        outs = [r["out"] for r in res.results]
        finite = all(np.isfinite(o).all() for o in outs)
        if finite and max(float(np.abs(o).max()) for o in outs) < 1e4:
            break
    full = np.stack(
        [np.concatenate([outs[2 * b], outs[2 * b + 1]], axis=0) for b in range(B)]
    )
    return full.astype(np.float32)



# revision 13
# speedup vs baseline: 1.1241x; 1.1241x over previous
"""MultiHeadCredibilityAttention TRN2 kernel (v2).

Sharding: 8 cores = (batch b, query-half qh). Each core computes K/V for its
full batch sequence (S=2048) and attention outputs for its 1024 queries.
Output slices concatenate to the full (4, 2048, 256) result — no collectives.

v2 strategy (vs baseline):
  - Scores as per-head K=32 matmuls (PE cost is out-columns, independent of
    contraction), f32r inputs at 1 cycle/col.
  - exp split across ACT (true exp) + DVE (Schraudolph bit-trick exp emitting
    bf16 directly: affine f32->int16 convert, bitcast to bf16). GPSIMD cannot
    touch PSUM on this toolchain, so it only does SBUF memsets + DMA issue.
  - attn@V flipped: exp(scores) [keys x 128q] is the PE stationary, V+ones
    [keys x 33] the moving operand -> 33 cols/instr instead of 512, with the
    softmax denominator produced by the ones column. ctx accumulates over all
    16 key tiles in PSUM.
  - K/Q/O biases folded into the PE via rank-1 (ones-row) accumulating
    matmuls; K^T/Q^T evacuate PSUM->SBUF by DMA (f32r, no convert pass);
    O-projection DMAs PSUM->DRAM directly.
"""

import os
import sys
from collections import deque

import numpy as np

sys.path.insert(0, "/opt/trn_rl_repo")

import concourse.bass as bass  # noqa: E402
import concourse.mybir as mybir  # noqa: E402
from concourse.tile import TileContext  # noqa: E402
from concourse import bass_utils  # noqa: E402

B, S, D, H, HD = 4, 2048, 256, 8, 32
SQ = S // 2  # queries per core
N_CORES = 8
NM = S // 128  # key tiles
NQT = SQ // 128  # query sub-tiles
F32 = mybir.dt.float32
F32R = mybir.dt.float32r
BF16 = mybir.dt.bfloat16
I16 = mybir.dt.int16
INV_SCALE = 1.0 / np.sqrt(HD)
VW = HD + 1  # V-block width: 32 dims + ones column

# exp engine split: counts over the 128 (head, key-tile) exp jobs
EXP_ACT = int(os.environ.get("K_EXP_ACT", "77"))
EXP_DVE = 128 - EXP_ACT
# Schraudolph constant (bf16 target): exp(x) ~ bf16_bits(x*128/ln2 + 127*128 - C)
SCH_C = float(os.environ.get("K_SCH_C", "8.0"))
SCH_A = float(128.0 / np.log(2.0)) * INV_SCALE
SCH_B = float(127 * 128 - SCH_C)


def _exp_pattern():
    """Deterministic largest-remainder interleave of A/D exp jobs."""
    counts = {"A": EXP_ACT, "D": EXP_DVE}
    total = sum(counts.values())
    assert total == H * NM
    acc = {k: 0.0 for k in counts}
    pat = []
    for _ in range(total):
        for k in counts:
            acc[k] += counts[k] / total
        pick = max(acc, key=lambda k: acc[k])
        acc[pick] -= 1.0
        pat.append(pick)
    return pat


def split_multiwaits(nc, max_waits=1):
    """This toolchain's walrus rejects >1 sync-wait per instruction; split
    extras into preceding single-wait NOPs on the same engine."""
    n = 0
    for f in nc.m.functions:
        for bb in f.blocks:
            out = []
            for ins in bb.instructions:
                si = ins.sync_info
                if (
                    si is not None
                    and si.on_wait is not None
                    and len(si.on_wait) > max_waits
                ):
                    waits = list(si.on_wait)
                    for j, w in enumerate(waits[:-max_waits]):
                        n += 1
                        out.append(
                            mybir.InstNoOp(
                                name=f"{ins.name}-wsplit{j}",
                                opcode="NoOp",
                                engine=ins.engine,
                                sync_info=mybir.SyncInfo(on_wait=[w], on_update=[]),
                            )
                        )
                    ins.sync_info = mybir.SyncInfo(
                        on_wait=waits[-max_waits:], on_update=list(si.on_update)
                    )
                out.append(ins)
            bb.instructions = out
    return n


def build_module():
    nc = bass.Bass("TRN2")
    xT_d = nc.dram_tensor("xT", [D, S], F32R, kind="ExternalInput")
    xTq_d = nc.dram_tensor("xTq", [D, SQ], F32R, kind="ExternalInput")
    wqT_d = nc.dram_tensor("wqT", [D, D], F32R, kind="ExternalInput")
    wkT_d = nc.dram_tensor("wkT", [D, D], F32R, kind="ExternalInput")
    wvT_d = nc.dram_tensor("wvT", [D, D], F32R, kind="ExternalInput")
    woT_d = nc.dram_tensor("woT", [D, D], BF16, kind="ExternalInput")
    bq_d = nc.dram_tensor("bq", [D, 1], F32, kind="ExternalInput")
    bk_d = nc.dram_tensor("bk", [D, 1], F32, kind="ExternalInput")
    bor_d = nc.dram_tensor("bor", [1, D], BF16, kind="ExternalInput")
    bvb_d = nc.dram_tensor("bvb", [128, D], F32, kind="ExternalInput")
    ident_d = nc.dram_tensor("ident", [128, 128], F32, kind="ExternalInput")
    out_d = nc.dram_tensor("out", [SQ, D], F32, kind="ExternalOutput")
    DEBUG = os.environ.get("K_DEBUG", "0") == "1"
    if DEBUG:
        dKT_d = nc.dram_tensor("dKT", [2, 128, S], BF16, kind="ExternalOutput")
        dQT_d = nc.dram_tensor("dQT", [2, 128, SQ], BF16, kind="ExternalOutput")
        dVA_d = nc.dram_tensor("dVA", [NM, 128, H * VW], BF16, kind="ExternalOutput")
        dCN_d = nc.dram_tensor("dCN", [NQT, 128, D], F32, kind="ExternalOutput")
        dCT_d = nc.dram_tensor("dCT", [2, 128, SQ], BF16, kind="ExternalOutput")

    pat = _exp_pattern()

    with TileContext(nc) as tc:
        with (
            tc.tile_pool(name="const", bufs=1) as cpool,
            tc.tile_pool(name="pers", bufs=1) as pers,
        ):
            # ---- load inputs ----
            w_sb = {}
            for nm, dram in (("q", wqT_d), ("k", wkT_d), ("v", wvT_d), ("o", woT_d)):
                wdt = BF16 if nm == "o" else F32R
                w_sb[nm] = [
                    cpool.tile([128, D], wdt, tag=f"w{nm}{d}", name=f"w{nm}{d}")
                    for d in range(2)
                ]
                for d in range(2):
                    nc.sync.dma_start(
                        out=w_sb[nm][d][:], in_=dram[d * 128 : (d + 1) * 128, :]
                    )
            xT_sb = [
                cpool.tile([128, S], F32R, tag=f"xT{d}", name=f"xT{d}")
                for d in range(2)
            ]
            xTq_sb = [
                cpool.tile([128, SQ], F32R, tag=f"xTq{d}", name=f"xTq{d}")
                for d in range(2)
            ]
            for d in range(2):
                nc.sync.dma_start(out=xT_sb[d][:], in_=xT_d[d * 128 : (d + 1) * 128, :])
                nc.sync.dma_start(
                    out=xTq_sb[d][:], in_=xTq_d[d * 128 : (d + 1) * 128, :]
                )
            bq_sb = [
                cpool.tile([128, 1], F32, tag=f"bq{d}", name=f"bq{d}") for d in range(2)
            ]
            bk_sb = [
                cpool.tile([128, 1], F32, tag=f"bk{d}", name=f"bk{d}") for d in range(2)
            ]
            for d in range(2):
                nc.sync.dma_start(out=bq_sb[d][:], in_=bq_d[d * 128 : (d + 1) * 128, :])
                nc.sync.dma_start(out=bk_sb[d][:], in_=bk_d[d * 128 : (d + 1) * 128, :])
            bor_sb = cpool.tile([1, D], BF16, tag="bor", name="bor")
            bvb_sb = cpool.tile([128, D], F32, tag="bvb", name="bvb")
            ident_sb = cpool.tile([128, 128], F32, tag="ident", name="ident")
            onesr_sb = cpool.tile([1, 512], BF16, tag="onesr", name="onesr")
            nc.sync.dma_start(out=bor_sb[:], in_=bor_d[:])
            nc.sync.dma_start(out=bvb_sb[:], in_=bvb_d[:])
            nc.sync.dma_start(out=ident_sb[:], in_=ident_d[:])
            nc.gpsimd.memset(onesr_sb[:], 1.0)

            # ---- persistent intermediates ----
            KT_sb = [
                pers.tile([128, S], BF16, tag=f"KT{t}", name=f"KT{t}") for t in range(2)
            ]
            QT_sb = [
                pers.tile([128, SQ], BF16, tag=f"QT{t}", name=f"QT{t}")
                for t in range(2)
            ]
            VA_sb = [
                pers.tile([128, H * VW], BF16, tag=f"VA{m}", name=f"VA{m}")
                for m in range(NM)
            ]
            ctxn_sb = [
                pers.tile([128, D], F32, tag=f"ctxn{qt}", name=f"ctxn{qt}")
                for qt in range(NQT)
            ]
            ctxT_sb = [
                pers.tile([128, SQ], BF16, tag=f"ctxT{d}", name=f"ctxT{d}")
                for d in range(2)
            ]

            # ---- projections (f32r; biases folded in as rank-1 matmuls) ----
            with tc.tile_pool(name="ps1", bufs=1, space="PSUM") as ps1:
                # K^T and Q^T: head dims on partitions (half t = heads 4t..4t+3)
                for half in range(2):
                    for kn in range(S // 512):
                        pk = ps1.tile([128, 512], F32, tag="pk", name="pk", bufs=3)
                        for d in range(2):
                            nc.tensor.matmul(
                                pk[:],
                                lhsT=w_sb["k"][d][:, half * 128 : (half + 1) * 128],
                                rhs=xT_sb[d][:, kn * 512 : (kn + 1) * 512],
                                start=(d == 0),
                                stop=(d == 1),
                            )
                        nc.scalar.activation(
                            KT_sb[half][:, kn * 512 : (kn + 1) * 512],
                            pk[:],
                            mybir.ActivationFunctionType.Identity,
                            bias=bk_sb[half][:, 0:1],
                        )
                    for qn in range(SQ // 512):
                        pq = ps1.tile([128, 512], F32, tag="pk", name="pq", bufs=3)
                        for d in range(2):
                            nc.tensor.matmul(
                                pq[:],
                                lhsT=w_sb["q"][d][:, half * 128 : (half + 1) * 128],
                                rhs=xTq_sb[d][:, qn * 512 : (qn + 1) * 512],
                                start=(d == 0),
                                stop=(d == 1),
                            )
                        nc.scalar.activation(
                            QT_sb[half][:, qn * 512 : (qn + 1) * 512],
                            pq[:],
                            mybir.ActivationFunctionType.Identity,
                            bias=bq_sb[half][:, 0:1],
                        )
                # V token-major, head blocks of 33 (32 dims + ones col)
                for m in range(NM):
                    pv = ps1.tile([128, D], F32, tag="pv", name="pv", bufs=3)
                    for d in range(2):
                        nc.tensor.matmul(
                            pv[:],
                            lhsT=xT_sb[d][:, m * 128 : (m + 1) * 128],
                            rhs=w_sb["v"][d][:],
                            start=(d == 0),
                            stop=(d == 1),
                        )
                    va = VA_sb[m][:].rearrange("p (h c) -> p h c", c=VW)
                    nc.vector.tensor_add(
                        va[:, :, 0:HD],
                        pv[:].rearrange("p (h c) -> p h c", c=HD),
                        bvb_sb[:].rearrange("p (h c) -> p h c", c=HD),
                    )
                    nc.gpsimd.memset(va[:, :, HD : HD + 1], 1.0)

            if DEBUG:
                for t in range(2):
                    nc.sync.dma_start(out=dKT_d[t], in_=KT_sb[t][:])
                    nc.sync.dma_start(out=dQT_d[t], in_=QT_sb[t][:])
                for m in range(NM):
                    nc.sync.dma_start(out=dVA_d[m], in_=VA_sb[m][:])

            # ---- attention: one head per pass, flash over 16 key tiles ----
            job = 0
            with (
                tc.tile_pool(name="ps_s", bufs=3, space="PSUM") as ps_s,
                tc.tile_pool(name="ps_c", bufs=2, space="PSUM") as ps_c,
                tc.tile_pool(name="expp", bufs=1) as expp,
            ):
                for h in range(H):
                    t, r = h // 4, (h % 4) * 32
                    KTh = KT_sb[t][r : r + 32, :]
                    QTh = QT_sb[t][r : r + 32, :]
                    pctx = ps_c.tile(
                        [128, NQT * VW], F32, tag="pc", name="pctx", bufs=2
                    )
                    nc.vector.memset(pctx[:], 0.0)
                    pend = deque()
                    for m in range(NM):
                        psc = ps_s.tile([128, SQ], F32, tag="s", name="psc", bufs=3)
                        for qn in range(SQ // 512):
                            nc.tensor.matmul(
                                psc[:, qn * 512 : (qn + 1) * 512],
                                lhsT=KTh[:, m * 128 : (m + 1) * 128],
                                rhs=QTh[:, qn * 512 : (qn + 1) * 512],
                                start=True,
                                stop=True,
                                tile_position=(r, 0),
                            )
                        ets = expp.tile([128, SQ], BF16, tag="ets", name="ets", bufs=5)
                        eng = pat[job]
                        job += 1
                        if eng == "A":
                            nc.scalar.activation(
                                ets[:],
                                psc[:],
                                mybir.ActivationFunctionType.Exp,
                                scale=float(INV_SCALE),
                            )
                        else:
                            nc.vector.tensor_scalar(
                                ets[:].bitcast(I16),
                                psc[:],
                                float(SCH_A),
                                float(SCH_B),
                                op0=mybir.AluOpType.mult,
                                op1=mybir.AluOpType.add,
                            )
                        pend.append((m, ets))
                        if len(pend) > 2:
                            ma, ea = pend.popleft()
                            for qt in range(NQT):
                                nc.tensor.matmul(
                                    pctx[:, qt * VW : (qt + 1) * VW],
                                    lhsT=ea[:, qt * 128 : (qt + 1) * 128],
                                    rhs=VA_sb[ma][:, h * VW : (h + 1) * VW],
                                    start=False,
                                    stop=(ma == NM - 1),
                                    skip_group_check=True,
                                )
                    while pend:
                        ma, ea = pend.popleft()
                        for qt in range(NQT):
                            nc.tensor.matmul(
                                pctx[:, qt * VW : (qt + 1) * VW],
                                lhsT=ea[:, qt * 128 : (qt + 1) * 128],
                                rhs=VA_sb[ma][:, h * VW : (h + 1) * VW],
                                start=False,
                                stop=(ma == NM - 1),
                                skip_group_check=True,
                            )
                    # normalize: batched reciprocal of the 8 qt denominators
                    rec = expp.tile([128, NQT], F32, tag="rec", name="rec", bufs=2)
                    nc.vector.reciprocal(rec[:], pctx[:, HD::VW])
                    for qt in range(NQT):
                        nc.vector.tensor_scalar_mul(
                            ctxn_sb[qt][:, h * HD : (h + 1) * HD],
                            pctx[:, qt * VW : qt * VW + HD],
                            rec[:, qt : qt + 1],
                        )

            if DEBUG:
                for qt in range(NQT):
                    nc.sync.dma_start(out=dCN_d[qt], in_=ctxn_sb[qt][:])

            # ---- transpose ctx and output projection ----
            with (
                tc.tile_pool(name="ps_t", bufs=1, space="PSUM") as ps_t,
                tc.tile_pool(name="ps_o", bufs=1, space="PSUM") as ps_o,
            ):
                for dh in range(2):
                    for qt in range(NQT):
                        pt = ps_t.tile([128, 128], F32, tag="pt", name="pt", bufs=4)
                        nc.tensor.transpose(
                            pt[:],
                            ctxn_sb[qt][:, dh * 128 : (dh + 1) * 128],
                            ident_sb[:],
                        )
                        nc.vector.tensor_copy(
                            ctxT_sb[dh][:, qt * 128 : (qt + 1) * 128],
                            pt[:],
                        )
                with tc.tile_pool(name="outp", bufs=1) as outp:
                    for qt in range(NQT):
                        po = ps_o.tile([128, D], F32, tag="po", name="po", bufs=2)
                        for d in range(2):
                            nc.tensor.matmul(
                                po[:],
                                lhsT=ctxT_sb[d][:, qt * 128 : (qt + 1) * 128],
                                rhs=w_sb["o"][d][:],
                                start=(d == 0),
                                stop=False,
                            )
                        nc.tensor.matmul(
                            po[:],
                            lhsT=onesr_sb[:, 0:128],
                            rhs=bor_sb[:],
                            start=False,
                            stop=True,
                        )
                        ot = outp.tile([128, D], F32, tag="ot", name="ot", bufs=2)
                        nc.scalar.activation(
                            ot[:], po[:], mybir.ActivationFunctionType.Copy
                        )
                        nc.sync.dma_start(
                            out=out_d[qt * 128 : (qt + 1) * 128, :], in_=ot[:]
                        )

    if DEBUG:
        for dh in range(2):
            nc_dummy = None  # ctxT dump handled below pools if needed
    split_multiwaits(nc)
    return nc


_module_cache = {}


def _get_module():
    key = (EXP_ACT, SCH_C)
    if key not in _module_cache:
        _module_cache[key] = build_module()
    return _module_cache[key]


def make_in_maps(inputs):
    import ml_dtypes

    _bf16 = np.dtype(ml_dtypes.bfloat16)
    x = np.asarray(inputs["x"], np.float32)
    Wq, bq = np.asarray(inputs["Wq"], np.float32), np.asarray(inputs["bq"], np.float32)
    Wk, bk = np.asarray(inputs["Wk"], np.float32), np.asarray(inputs["bk"], np.float32)
    Wv, bv = np.asarray(inputs["Wv"], np.float32), np.asarray(inputs["bv"], np.float32)
    Wo, bo = np.asarray(inputs["Wo"], np.float32), np.asarray(inputs["bo"], np.float32)
    hs = np.asarray(inputs["head_scale"], np.float32)

    hs_col = np.repeat(hs, HD)  # head_scale folded into V
    shared = {
        "wqT": np.ascontiguousarray(Wq.T),
        "wkT": np.ascontiguousarray(Wk.T),
        "wvT": np.ascontiguousarray((Wv * hs_col[:, None]).T),
        "woT": np.ascontiguousarray(Wo.T.astype(_bf16)),
        "bq": np.ascontiguousarray(bq.reshape(D, 1)),
        "bk": np.ascontiguousarray(bk.reshape(D, 1)),
        "bor": np.ascontiguousarray(bo.reshape(1, D).astype(_bf16)),
        "bvb": np.ascontiguousarray(
            np.broadcast_to(bv * hs_col, (128, D)).astype(np.float32)
        ),
        "ident": np.eye(128, dtype=np.float32),
    }
    in_maps = []
    for k in range(N_CORES):
        b, qh = k // 2, k % 2
        xT = np.ascontiguousarray(x[b].T)
        in_maps.append(
            {
                "xT": xT,
                "xTq": np.ascontiguousarray(xT[:, qh * SQ : (qh + 1) * SQ]),
                **shared,
            }
        )
    return in_maps


def kernel(x, Wq, bq, Wk, bk, Wv, bv, Wo, bo, head_scale):
    in_maps = make_in_maps(
        dict(
            x=x, Wq=Wq, bq=bq, Wk=Wk, bk=bk, Wv=Wv, bv=bv, Wo=Wo, bo=bo,
            head_scale=head_scale,
        )
    )
    nc = _get_module()
    core_ids = list(range(N_CORES))
    # First execution after a fresh process attach has been observed to
    # return corrupted results on some cores; run once to warm up, then
    # use the second run (retry if it still looks corrupted).
    bass_utils.run_bass_kernel_spmd(nc, in_maps, core_ids=core_ids)
    for _ in range(3):
        res = bass_utils.run_bass_kernel_spmd(nc, in_maps, core_ids=core_ids)
        outs = [r["out"] for r in res.results]
        finite = all(np.isfinite(o).all() for o in outs)
        if finite and max(float(np.abs(o).max()) for o in outs) < 1e4:
            break
    full = np.stack(
        [np.concatenate([outs[2 * b], outs[2 * b + 1]], axis=0) for b in range(B)]
    )
    return full.astype(np.float32)
